# revision 20
# baseline (speedup 1.0000x reference)
"""HypergraphConv (node->edge->node message passing) on 8 Trainium2 NeuronCores.

Self-contained Trainium kernel for:
    xw   = x @ W
    m_e  = (1/deg_e) * sum_{k: edge[k]=e} xw[src[k]]
    o_i  = (1/deg_i) * sum_{k: src[k]=i} m_{edge[k]} + bias
    out  = mean_i relu(o_i)                       # [128]

Sharding: nodes are split across the 8 cores (6250 each). Each core owns the
incidence entries whose src node falls in its shard; those entries drive both
the node->edge scatter (partial m, ReduceScattered + AllGathered across cores)
and the edge->node scatter (complete rows for the core's nodes).

Scatters are one-hot matmuls over sorted-and-padded entry streams grouped into
128-wide edge tiles / 64-wide node tiles; gathers use the SWDGE dma_gather
engine against bf16 row tables (256-byte rows). Degrees are computed
host-side and shipped as small inputs. The phase-2 matmul is transposed
(out = G^T-style [feat, node]) so bias+ReLU fuse into one Activation-engine op
writing straight into the accumulation buffer.
"""

import os
import numpy as np
import ml_dtypes
from contextlib import ExitStack

import concourse.bacc as bacc
import concourse.bass as bass
import concourse.mybir as mybir
import concourse.tile as tile
from concourse import library_config
from concourse.bass_utils import run_bass_kernel_spmd

NCORES = 8
P = 128

N_NODES = 50000
N_EDGES = 20000
IN_DIM = 256
OUT_DIM = 128

ETW = 128          # edge-tile (group) width for phase-1 scatter
NTW = 64           # node-tile width for phase-2 scatter
SUPER = 48         # max 128-entry chunks per dma_gather call

BF16 = mybir.dt.bfloat16
F32 = mybir.dt.float32
I16 = mybir.dt.int16

PAD_OH = 200.0  # one-hot index for padding entries: matches no iota column


def _derived():
    npc = N_NODES // NCORES
    n_node_groups = (npc + NTW - 1) // NTW
    quantum = NCORES * P
    erows = -(-N_EDGES // quantum) * quantum  # RS-shardable, mult of 128
    n_edge_groups = erows // ETW
    return npc, n_node_groups, n_edge_groups, erows, erows // NCORES


def _wrap_idx16(idx):
    """[L] int -> [128, L//16] int16 SWDGE index layout (16-wrap, x8 replicas)."""
    a = np.asarray(idx, dtype=np.int16).reshape(-1, 16).T
    return np.ascontiguousarray(np.tile(a, (8, 1)))


def _oh_cols(oh):
    """[L] float -> [128, L//128] bf16: column c holds entries c*128..c*128+127."""
    return np.ascontiguousarray(oh.reshape(-1, P).T.astype(ml_dtypes.bfloat16))


def _bucket_entries(gidx, ohval, tid, n_tiles, chunks):
    """Lay out (gather idx, one-hot) entry streams grouped by tile.

    chunks[t]: number of 128-entry chunks allotted to tile t (static, shared
    across cores). Pad gather idx = 0 (contribution killed by the all-zero
    one-hot row). Returns (gather_idx[L], onehot[L]).
    """
    # sort by (tile, gather idx): idx-sorted slots give the SWDGE's 16-idx
    # descriptors HBM row-buffer locality; slot order is free (one-hot maps it)
    order = np.lexsort((gidx, tid))
    gidx = gidx[order]
    ohval = ohval[order]
    tid_s = tid[order]
    counts = np.bincount(tid_s, minlength=n_tiles)
    starts = np.concatenate([[0], np.cumsum(counts[:-1])])
    dest_base = np.concatenate([[0], np.cumsum(chunks[:-1])]) * P
    L = int(chunks.sum()) * P
    g_out = np.zeros(L, dtype=np.int64)
    oh_out = np.full(L, PAD_OH, dtype=np.float32)
    n = gidx.shape[0]
    rank = np.arange(n, dtype=np.int64) - starts[tid_s]
    dest = dest_base[tid_s] + rank
    g_out[dest] = gidx
    oh_out[dest] = ohval
    return g_out, oh_out


def build_kernel(chunks1, chunks2):
    """Build the SPMD device program.

    chunks1[g]: #chunks for edge group g (phase 1, may be 0);
    chunks2[gg]: #chunks for node group gg (phase 2, >= 1).
    """
    npc, n_node_groups, n_edge_groups, EROWS, SHARD = _derived()
    NROWS = n_node_groups * NTW
    LA = int(np.sum(chunks1)) * P
    LB = int(np.sum(chunks2)) * P
    NCA = LA // P
    NCB = LB // P
    REPS = int(os.environ.get("KREPS", "1"))
    no_gather = os.environ.get("DBG_NO_GATHER") == "1"  # timing bisection only
    no_scatmm = os.environ.get("DBG_NO_SCATMM") == "1"
    single_packet = os.environ.get("KSP", "0") == "1"

    nc = bacc.Bacc("TRN2", num_devices=NCORES)

    xT_in = nc.dram_tensor("xT", [IN_DIM, npc], BF16, kind="ExternalInput")
    w_in = nc.dram_tensor("w", [IN_DIM, OUT_DIM], BF16, kind="ExternalInput")
    biasT_in = nc.dram_tensor("biasT", [OUT_DIM, 1], F32, kind="ExternalInput")
    dinv_in = nc.dram_tensor("dinv", [1, NROWS], F32, kind="ExternalInput")
    binv_in = nc.dram_tensor("binv", [P, SHARD // P], F32, kind="ExternalInput")
    idxA_in = nc.dram_tensor("idxA", [P, LA // 16], I16, kind="ExternalInput")
    ohA_in = nc.dram_tensor("ohA", [P, NCA], BF16, kind="ExternalInput")
    idxB_in = nc.dram_tensor("idxB", [P, LB // 16], I16, kind="ExternalInput")
    ohB_in = nc.dram_tensor("ohB", [P, NCB], BF16, kind="ExternalInput")
    out_part = nc.dram_tensor("out_part", [OUT_DIM, 1], F32, kind="ExternalOutput")

    xwhl = nc.dram_tensor("xwhl", [npc, OUT_DIM], BF16)
    m_part = nc.dram_tensor("m_part", [EROWS, OUT_DIM], BF16)
    m_shard = nc.dram_tensor("m_shard", [SHARD, OUT_DIM], BF16)
    mtab_sh = nc.dram_tensor("mtab_sh", [SHARD, OUT_DIM], BF16)
    mtab = nc.dram_tensor("mtab", [EROWS, OUT_DIM], BF16, addr_space="Shared")

    # phase-1 super-groups: consecutive edge groups gathered in one call
    def make_supers(chunks):
        supers = []  # (chunk_offset, [(tile_idx, kt, local_chunk_off)])
        cur, ck, coff = [], 0, 0
        base = 0
        for t, k in enumerate(chunks):
            k = int(k)
            if k == 0:
                continue
            if ck + k > SUPER and cur:
                supers.append((coff, cur))
                coff += ck
                cur, ck = [], 0
            cur.append((t, k, ck))
            ck += k
        if cur:
            supers.append((coff, cur))
        return supers

    supers1 = make_supers(chunks1)
    supers2 = make_supers(chunks2)
    empty1 = [t for t, k in enumerate(chunks1) if int(k) == 0]

    with tile.TileContext(nc) as tc, ExitStack() as ctx:
        pin = ctx.enter_context(tc.tile_pool(name="pin", bufs=1))

        nc.gpsimd.load_library(library_config.mlp)

        # ---- persistent small tiles (once, outside reps) ----------------
        iota_i = pin.tile([P, P], I16)
        iota_bf = pin.tile([P, P], BF16)
        nc.gpsimd.iota(iota_i[:], [[1, P]], channel_multiplier=0)
        nc.vector.tensor_copy(out=iota_bf[:], in_=iota_i[:])

        def s_build(S_tile, oh_tile, col0, k, w):
            """S[p, c*w+j] = (oh[p, col0+c] == j), one DVE op for k chunks."""
            s_ap = S_tile[:].rearrange("p (k j) -> p k j", k=k)
            o = oh_tile[:, col0:col0 + k]
            in0 = bass.AP(o.tensor, o.offset, [list(o.ap[0]), list(o.ap[1]), [0, w]])
            it = iota_bf[:]
            in1 = bass.AP(it.tensor, it.offset, [list(it.ap[0]), [0, k], [1, w]])
            nc.vector.tensor_tensor(out=s_ap, in0=in0, in1=in1, op=mybir.AluOpType.is_equal)

        for rep in range(REPS):
          with tc.tile_pool(name=f"prep{rep}", bufs=1) as pr:
            # streams + per-rep persistent tiles
            idxA = pr.tile([P, LA // 16], I16, name="idxA")
            ohA = pr.tile([P, NCA], BF16, name="ohA")
            idxB = pr.tile([P, LB // 16], I16, name="idxB")
            ohB = pr.tile([P, NCB], BF16, name="ohB")
            nc.sync.dma_start(out=idxA[:], in_=idxA_in[:])
            nc.sync.dma_start(out=ohA[:], in_=ohA_in[:])
            nc.sync.dma_start(out=idxB[:], in_=idxB_in[:])
            nc.sync.dma_start(out=ohB[:], in_=ohB_in[:])
            bias_sb = pr.tile([P, 1], F32, name="biasT")
            nc.sync.dma_start(out=bias_sb[:OUT_DIM], in_=biasT_in[:])
            dinv_bc = pr.tile([P, NROWS], F32, name="dinvbc")
            nc.sync.dma_start(
                out=dinv_bc[:], in_=bass.AP(dinv_in, 0, [[0, P], [1, NROWS]]))
            binv_sb = pr.tile([P, SHARD // P], F32, name="binv")
            nc.sync.dma_start(out=binv_sb[:], in_=binv_in[:])
            accT = pr.tile([P, NROWS], F32, name="accT")

            # ---- stage A: xw = x @ W -> bf16 row table ------------------
            with tc.tile_pool(name=f"pa{rep}", bufs=1) as pa, \
                 tc.tile_pool(name=f"pa2{rep}", bufs=3) as pa2, \
                 tc.tile_pool(name=f"psa{rep}", bufs=2, space="PSUM") as psa:
                kh = IN_DIM // P
                xT_sb = [pa.tile([P, npc], BF16, name=f"xT{k}") for k in range(kh)]
                w_sb = [pa.tile([P, OUT_DIM], BF16, name=f"wsb{k}") for k in range(kh)]
                for k in range(kh):
                    nc.sync.dma_start(out=xT_sb[k][:], in_=xT_in[k * P:(k + 1) * P, :])
                    nc.sync.dma_start(out=w_sb[k][:], in_=w_in[k * P:(k + 1) * P, :])
                for i in range(0, npc, P):
                    nt = min(P, npc - i)
                    pxw = psa.tile([P, OUT_DIM], F32, tag="pxw")
                    for k in range(kh):
                        nc.tensor.matmul(
                            out=pxw[:nt], lhsT=xT_sb[k][:, i:i + nt], rhs=w_sb[k][:],
                            start=(k == 0), stop=(k == kh - 1))
                    xst = pa2.tile([P, OUT_DIM], BF16, tag="xst")
                    nc.scalar.activation(
                        out=xst[:nt], in_=pxw[:nt],
                        func=mybir.ActivationFunctionType.Copy)
                    nc.sync.dma_start(out=xwhl[i:i + nt, :], in_=xst[:nt, :])

            # ---- stage B: phase-1 scatter (node -> edge) ----------------
            with tc.tile_pool(name=f"pb{rep}", bufs=3) as pb, \
                 tc.tile_pool(name=f"psb{rep}", bufs=2, space="PSUM") as psb:
                zrow = pb.tile([P, OUT_DIM], BF16, tag="zrow", name="zrow")
                nc.vector.memset(zrow[:], 0.0)
                for t in empty1:
                    nc.sync.dma_start(
                        out=m_part[t * ETW:(t + 1) * ETW, :], in_=zrow[:ETW, :])
                for coff, groups in supers1:
                    ck_tot = sum(k for _, k, _ in groups)
                    G = pb.tile([P, ck_tot, OUT_DIM], BF16, tag="G")
                    if not no_gather:
                        nc.gpsimd.dma_gather(
                            G[:, :, :], xwhl[:, :],
                            idxA[:, coff * 8:(coff + ck_tot) * 8],
                            ck_tot * P, ck_tot * P, OUT_DIM,
                            single_packet=single_packet)
                    else:
                        nc.vector.memset(G[:, 0, :], 0.0)
                    for t, kt, loc in groups:
                        S = pb.tile([P, kt * ETW], BF16, tag="S")
                        s_build(S, ohA, coff + loc, kt, ETW)
                        pm = psb.tile([P, OUT_DIM], F32, tag="pm")
                        nkt = 1 if no_scatmm else kt
                        for c in range(nkt):
                            nc.tensor.matmul(
                                out=pm[:ETW], lhsT=S[:, c * ETW:(c + 1) * ETW],
                                rhs=G[:, loc + c, :],
                                start=(c == 0), stop=(c == nkt - 1),
                                skip_group_check=True)
                        mt = pb.tile([P, OUT_DIM], BF16, tag="mt")
                        nc.scalar.activation(
                            out=mt[:ETW], in_=pm[:ETW],
                            func=mybir.ActivationFunctionType.Copy)
                        nc.sync.dma_start(
                            out=m_part[t * ETW:(t + 1) * ETW, :], in_=mt[:ETW, :])

            # ---- stage C: ReduceScatter -> scale -> AllGather -----------
            no_cc = os.environ.get("DBG_NO_CC") == "1"  # TimelineSim can't
            if no_cc:                                   # model collectives
                nc.sync.dma_start(out=m_shard[:, :], in_=m_part[:SHARD, :])
            else:
                nc.gpsimd.collective_compute(
                    "ReduceScatter", mybir.AluOpType.add,
                    replica_groups=[list(range(NCORES))],
                    ins=[m_part[:, :]], outs=[m_shard[:, :]])
            with tc.tile_pool(name=f"pc{rep}", bufs=3) as pc:
                for t in range(SHARD // P):
                    ms = pc.tile([P, OUT_DIM], BF16, tag="ms")
                    nc.sync.dma_start(out=ms[:], in_=m_shard[t * P:(t + 1) * P, :])
                    st = pc.tile([P, OUT_DIM], BF16, tag="st")
                    nc.vector.tensor_scalar(
                        out=st[:], in0=ms[:], scalar1=binv_sb[:, t:t + 1],
                        scalar2=None, op0=mybir.AluOpType.mult)
                    nc.sync.dma_start(out=mtab_sh[t * P:(t + 1) * P, :], in_=st[:])
            if no_cc:
                for cc in range(NCORES):
                    nc.sync.dma_start(
                        out=mtab[cc * SHARD:(cc + 1) * SHARD, :],
                        in_=mtab_sh[:, :])
            else:
                nc.gpsimd.collective_compute(
                    "AllGather", mybir.AluOpType.bypass,
                    replica_groups=[list(range(NCORES))],
                    ins=[mtab_sh[:, :]], outs=[mtab[:, :]])

            # ---- stage D: phase-2 scatter (edge -> node), transposed ----
            with tc.tile_pool(name=f"pd{rep}", bufs=3) as pd, \
                 tc.tile_pool(name=f"psd{rep}", bufs=2, space="PSUM") as psd:
                for coff, groups in supers2:
                    ck_tot = sum(k for _, k, _ in groups)
                    G2 = pd.tile([P, ck_tot, OUT_DIM], BF16, tag="G2")
                    if not no_gather:
                        nc.gpsimd.dma_gather(
                            G2[:, :, :], mtab[:, :],
                            idxB[:, coff * 8:(coff + ck_tot) * 8],
                            ck_tot * P, ck_tot * P, OUT_DIM,
                            single_packet=single_packet)
                    else:
                        nc.vector.memset(G2[:, 0, :], 0.0)
                    for gg, kt, loc in groups:
                        S2 = pd.tile([P, kt * NTW], BF16, tag="S2")
                        s_build(S2, ohB, coff + loc, kt, NTW)
                        poT = psd.tile([P, NTW], F32, tag="poT")
                        nkt = 1 if no_scatmm else kt
                        for c in range(nkt):
                            nc.tensor.matmul(
                                out=poT[:], lhsT=G2[:, loc + c, :],
                                rhs=S2[:, c * NTW:(c + 1) * NTW],
                                start=(c == 0), stop=(c == nkt - 1),
                                skip_group_check=True)
                        ot = pd.tile([P, NTW], F32, tag="ot")
                        nc.vector.tensor_tensor(
                            out=ot[:], in0=poT[:],
                            in1=dinv_bc[:, gg * NTW:(gg + 1) * NTW],
                            op=mybir.AluOpType.mult)
                        nc.scalar.activation(
                            out=accT[:, gg * NTW:(gg + 1) * NTW], in_=ot[:],
                            func=mybir.ActivationFunctionType.Relu,
                            bias=bias_sb[:, 0:1])
                # zero phantom-node columns (beyond npc) before the reduce
                if NROWS > npc:
                    nc.vector.memset(accT[:, npc:NROWS], 0.0)

            # ---- stage E: row-sum over all node columns -> [OUT_DIM, 1] -
            with tc.tile_pool(name=f"pe{rep}", bufs=1) as pe:
                osum = pe.tile([P, 1], F32)
                nc.vector.tensor_reduce(
                    out=osum[:], in_=accT[:], axis=mybir.AxisListType.X,
                    op=mybir.AluOpType.add)
                nc.sync.dma_start(out=out_part[:, :], in_=osum[:OUT_DIM])

    nc.compile()
    return nc


def prepare_inputs(x, w, bias, hyperedge_index):
    """Host-side sharding: split entries by src-node shard, bucket/pad both
    phase streams, compute degrees + static chunk structure (shared by all
    cores)."""
    npc, n_node_groups, n_edge_groups, EROWS, SHARD = _derived()
    NROWS = n_node_groups * NTW
    src = np.asarray(hyperedge_index[0], dtype=np.int64)
    edge = np.asarray(hyperedge_index[1], dtype=np.int64)

    # exact degree reciprocals (host)
    deg_e = np.bincount(edge, minlength=N_EDGES).astype(np.float64)
    b_inv_full = np.zeros(EROWS, np.float32)
    nzmask = deg_e > 0
    b_inv_full[:N_EDGES][nzmask] = (1.0 / deg_e[nzmask]).astype(np.float32)

    core_of = src // npc
    per_core = []
    for c in range(NCORES):
        sel = core_of == c
        per_core.append((src[sel] - c * npc, edge[sel]))

    cnt1 = np.zeros((NCORES, n_edge_groups), np.int64)
    cnt2 = np.zeros((NCORES, n_node_groups), np.int64)
    for c, (s_loc, e_glob) in enumerate(per_core):
        cnt1[c] = np.bincount(e_glob // ETW, minlength=n_edge_groups)
        cnt2[c] = np.bincount(s_loc // NTW, minlength=n_node_groups)
    chunks1 = -(-cnt1.max(axis=0) // P)                 # may be 0
    chunks2 = np.maximum(1, -(-cnt2.max(axis=0) // P))  # >= 1 (bias/relu rows)

    in_maps = []
    for c, (s_loc, e_glob) in enumerate(per_core):
        g1, oh1 = _bucket_entries(
            s_loc, (e_glob % ETW).astype(np.float32), e_glob // ETW,
            n_edge_groups, chunks1)
        g2, oh2 = _bucket_entries(
            e_glob, (s_loc % NTW).astype(np.float32), s_loc // NTW,
            n_node_groups, chunks2)

        deg_n = np.bincount(s_loc, minlength=npc).astype(np.float64)
        d_inv = np.zeros(NROWS, np.float32)
        nz = deg_n > 0
        d_inv[:npc][nz] = (1.0 / deg_n[nz]).astype(np.float32)

        binv_shard = np.ascontiguousarray(
            b_inv_full[c * SHARD:(c + 1) * SHARD].reshape(SHARD // P, P).T)

        xT = np.ascontiguousarray(
            x[c * npc:(c + 1) * npc].T.astype(ml_dtypes.bfloat16))
        in_maps.append({
            "xT": xT,
            "w": np.ascontiguousarray(w.astype(ml_dtypes.bfloat16)),
            "biasT": np.ascontiguousarray(bias.astype(np.float32)).reshape(-1, 1),
            "dinv": d_inv.reshape(1, -1),
            "binv": binv_shard,
            "idxA": _wrap_idx16(g1),
            "ohA": _oh_cols(oh1),
            "idxB": _wrap_idx16(g2),
            "ohB": _oh_cols(oh2),
        })

    return in_maps, chunks1, chunks2


def kernel(x_node_features, lin_weight, bias, hyperedge_index):
    in_maps, chunks1, chunks2 = prepare_inputs(
        x_node_features, lin_weight, bias, hyperedge_index)
    nc = build_kernel(chunks1, chunks2)
    res = run_bass_kernel_spmd(nc, in_maps, list(range(NCORES)))
    total = np.zeros(OUT_DIM, np.float64)
    for c in range(NCORES):
        total += res.results[c]["out_part"][:, 0].astype(np.float64)
    return (total / N_NODES).astype(np.float32)


# revision 25
# speedup vs baseline: 1.8735x; 1.8735x over previous
"""HypergraphConv (node->edge->node message passing) on 8 Trainium2 NeuronCores.

Self-contained Trainium kernel for:
    xw   = x @ W
    m_e  = (1/deg_e) * sum_{k: edge[k]=e} xw[src[k]]
    o_i  = (1/deg_i) * sum_{k: src[k]=i} m_{edge[k]} + bias
    out  = mean_i relu(o_i)                       # [128]

Sharding: nodes are split across the 8 cores (6250 each). Each core owns the
incidence entries whose src node falls in its shard; those entries drive both
the node->edge scatter (partial m, ReduceScattered + AllGathered across cores)
and the edge->node scatter (complete rows for the core's nodes).

Scatters are one-hot matmuls over sorted-and-padded entry streams grouped into
128-wide edge tiles / 64-wide node tiles; gathers use the SWDGE dma_gather
engine against bf16 row tables (256-byte rows). Degrees are computed
host-side and shipped as small inputs. The phase-2 matmul is transposed
(out = G^T-style [feat, node]) so bias+ReLU fuse into one Activation-engine op
writing straight into the accumulation buffer.
"""

import os
import numpy as np
import ml_dtypes
from contextlib import ExitStack

import concourse.bacc as bacc
import concourse.bass as bass
import concourse.mybir as mybir
import concourse.tile as tile
from concourse import library_config
from concourse.bass_utils import run_bass_kernel_spmd

NCORES = 8
P = 128

N_NODES = 50000
N_EDGES = 20000
IN_DIM = 256
OUT_DIM = 128

ETW = 128          # edge-tile (group) width for phase-1 scatter
NTW = 64           # node-tile width for phase-2 scatter
SUPER = int(os.environ.get("KSUPER", "48"))  # 128-entry chunks per gather call

BF16 = mybir.dt.bfloat16
F32 = mybir.dt.float32
I16 = mybir.dt.int16

PAD_OH = 200.0  # one-hot index for padding entries: matches no iota column


def _derived():
    npc = N_NODES // NCORES
    n_node_groups = (npc + NTW - 1) // NTW
    quantum = NCORES * P
    erows = -(-N_EDGES // quantum) * quantum  # RS-shardable, mult of 128
    n_edge_groups = erows // ETW
    return npc, n_node_groups, n_edge_groups, erows, erows // NCORES


def _wrap_idx16(idx):
    """[L] int -> [128, L//16] int16 SWDGE index layout (16-wrap, x8 replicas)."""
    a = np.asarray(idx, dtype=np.int16).reshape(-1, 16).T
    return np.ascontiguousarray(np.tile(a, (8, 1)))


def _oh_cols(oh):
    """[L] float -> [128, L//128] bf16: column c holds entries c*128..c*128+127."""
    return np.ascontiguousarray(oh.reshape(-1, P).T.astype(ml_dtypes.bfloat16))


def _bucket_entries(gidx, ohval, tid, n_tiles, chunks):
    """Lay out (gather idx, one-hot) entry streams grouped by tile.

    chunks[t]: number of 128-entry chunks allotted to tile t (static, shared
    across cores). Pad gather idx = 0 (contribution killed by the all-zero
    one-hot row). Returns (gather_idx[L], onehot[L]).
    """
    # sort by (tile, gather idx): idx-sorted slots give the SWDGE's 16-idx
    # descriptors HBM row-buffer locality; slot order is free (one-hot maps it)
    order = np.lexsort((gidx, tid))
    gidx = gidx[order]
    ohval = ohval[order]
    tid_s = tid[order]
    counts = np.bincount(tid_s, minlength=n_tiles)
    starts = np.concatenate([[0], np.cumsum(counts[:-1])])
    dest_base = np.concatenate([[0], np.cumsum(chunks[:-1])]) * P
    L = int(chunks.sum()) * P
    g_out = np.zeros(L, dtype=np.int64)
    oh_out = np.full(L, PAD_OH, dtype=np.float32)
    n = gidx.shape[0]
    rank = np.arange(n, dtype=np.int64) - starts[tid_s]
    dest = dest_base[tid_s] + rank
    g_out[dest] = gidx
    oh_out[dest] = ohval
    return g_out, oh_out


def build_kernel(chunks1, chunks2):
    """Build the SPMD device program.

    chunks1[g]: #chunks for edge group g (phase 1, may be 0);
    chunks2[gg]: #chunks for node group gg (phase 2, >= 1).
    """
    npc, n_node_groups, n_edge_groups, EROWS, SHARD = _derived()
    NROWS = n_node_groups * NTW
    LA = int(np.sum(chunks1)) * P
    LB = int(np.sum(chunks2)) * P
    NCA = LA // P
    NCB = LB // P
    REPS = int(os.environ.get("KREPS", "1"))
    no_gather = os.environ.get("DBG_NO_GATHER") == "1"  # timing bisection only
    no_scatmm = os.environ.get("DBG_NO_SCATMM") == "1"
    single_packet = os.environ.get("KSP", "0") == "1"
    scratch = int(os.environ.get("KSCRATCH", "32768"))
    nqueues = int(os.environ.get("KQUEUES", "4"))  # ucode MAX_SWDGE_QUEUES=4

    nc = bacc.Bacc("TRN2", num_devices=NCORES,
                   dynamic_dma_scratch_size=scratch,
                   num_swdge_queues=nqueues)

    xT_in = nc.dram_tensor("xT", [IN_DIM, npc], BF16, kind="ExternalInput")
    w_in = nc.dram_tensor("w", [IN_DIM, OUT_DIM], BF16, kind="ExternalInput")
    biasT_in = nc.dram_tensor("biasT", [OUT_DIM, 1], F32, kind="ExternalInput")
    dinv_in = nc.dram_tensor("dinv", [1, NROWS], F32, kind="ExternalInput")
    binv_in = nc.dram_tensor("binv", [P, SHARD // P], F32, kind="ExternalInput")
    idxA_in = nc.dram_tensor("idxA", [P, LA // 16], I16, kind="ExternalInput")
    ohA_in = nc.dram_tensor("ohA", [P, NCA], BF16, kind="ExternalInput")
    idxB_in = nc.dram_tensor("idxB", [P, LB // 16], I16, kind="ExternalInput")
    ohB_in = nc.dram_tensor("ohB", [P, NCB], BF16, kind="ExternalInput")
    out_part = nc.dram_tensor("out_part", [OUT_DIM, 1], F32, kind="ExternalOutput")

    xwhl = nc.dram_tensor("xwhl", [npc, OUT_DIM], BF16)
    m_part = nc.dram_tensor("m_part", [EROWS, OUT_DIM], BF16)
    m_shard = nc.dram_tensor("m_shard", [SHARD, OUT_DIM], BF16)
    mtab_sh = nc.dram_tensor("mtab_sh", [SHARD, OUT_DIM], BF16)
    mtab = nc.dram_tensor("mtab", [EROWS, OUT_DIM], BF16, addr_space="Shared")

    # phase-1 super-groups: consecutive edge groups gathered in one call
    def make_supers(chunks):
        supers = []  # (chunk_offset, [(tile_idx, kt, local_chunk_off)])
        cur, ck, coff = [], 0, 0
        base = 0
        for t, k in enumerate(chunks):
            k = int(k)
            if k == 0:
                continue
            if ck + k > SUPER and cur:
                supers.append((coff, cur))
                coff += ck
                cur, ck = [], 0
            cur.append((t, k, ck))
            ck += k
        if cur:
            supers.append((coff, cur))
        return supers

    supers1 = make_supers(chunks1)
    supers2 = make_supers(chunks2)
    empty1 = [t for t, k in enumerate(chunks1) if int(k) == 0]

    with tile.TileContext(nc) as tc, ExitStack() as ctx:
        pin = ctx.enter_context(tc.tile_pool(name="pin", bufs=1))

        nc.gpsimd.load_library(library_config.mlp)

        # ---- persistent small tiles (once, outside reps) ----------------
        iota_i = pin.tile([P, P], I16)
        iota_bf = pin.tile([P, P], BF16)
        nc.gpsimd.iota(iota_i[:], [[1, P]], channel_multiplier=0)
        nc.vector.tensor_copy(out=iota_bf[:], in_=iota_i[:])

        def s_build(S_tile, oh_tile, col0, k, w):
            """S[p, c*w+j] = (oh[p, col0+c] == j), one DVE op for k chunks."""
            s_ap = S_tile[:].rearrange("p (k j) -> p k j", k=k)
            o = oh_tile[:, col0:col0 + k]
            in0 = bass.AP(o.tensor, o.offset, [list(o.ap[0]), list(o.ap[1]), [0, w]])
            it = iota_bf[:]
            in1 = bass.AP(it.tensor, it.offset, [list(it.ap[0]), [0, k], [1, w]])
            nc.vector.tensor_tensor(out=s_ap, in0=in0, in1=in1, op=mybir.AluOpType.is_equal)

        for rep in range(REPS):
          with tc.tile_pool(name=f"prep{rep}", bufs=1) as pr:
            # streams + per-rep persistent tiles
            idxA = pr.tile([P, LA // 16], I16, name="idxA")
            ohA = pr.tile([P, NCA], BF16, name="ohA")
            idxB = pr.tile([P, LB // 16], I16, name="idxB")
            ohB = pr.tile([P, NCB], BF16, name="ohB")
            nc.sync.dma_start(out=idxA[:], in_=idxA_in[:])
            nc.sync.dma_start(out=ohA[:], in_=ohA_in[:])
            nc.sync.dma_start(out=idxB[:], in_=idxB_in[:])
            nc.sync.dma_start(out=ohB[:], in_=ohB_in[:])
            bias_sb = pr.tile([P, 1], F32, name="biasT")
            nc.sync.dma_start(out=bias_sb[:OUT_DIM], in_=biasT_in[:])
            dinv_bc = pr.tile([P, NROWS], F32, name="dinvbc")
            nc.sync.dma_start(
                out=dinv_bc[:], in_=bass.AP(dinv_in, 0, [[0, P], [1, NROWS]]))
            binv_sb = pr.tile([P, SHARD // P], F32, name="binv")
            nc.sync.dma_start(out=binv_sb[:], in_=binv_in[:])
            accT = pr.tile([P, NROWS], F32, name="accT")

            # ---- stage A: xw = x @ W -> bf16 row table ------------------
            with tc.tile_pool(name=f"pa{rep}", bufs=1) as pa, \
                 tc.tile_pool(name=f"pa2{rep}", bufs=3) as pa2, \
                 tc.tile_pool(name=f"psa{rep}", bufs=2, space="PSUM") as psa:
                kh = IN_DIM // P
                xT_sb = [pa.tile([P, npc], BF16, name=f"xT{k}") for k in range(kh)]
                w_sb = [pa.tile([P, OUT_DIM], BF16, name=f"wsb{k}") for k in range(kh)]
                for k in range(kh):
                    nc.sync.dma_start(out=xT_sb[k][:], in_=xT_in[k * P:(k + 1) * P, :])
                    nc.sync.dma_start(out=w_sb[k][:], in_=w_in[k * P:(k + 1) * P, :])
                for i in range(0, npc, P):
                    nt = min(P, npc - i)
                    pxw = psa.tile([P, OUT_DIM], F32, tag="pxw")
                    for k in range(kh):
                        nc.tensor.matmul(
                            out=pxw[:nt], lhsT=xT_sb[k][:, i:i + nt], rhs=w_sb[k][:],
                            start=(k == 0), stop=(k == kh - 1))
                    xst = pa2.tile([P, OUT_DIM], BF16, tag="xst")
                    nc.scalar.activation(
                        out=xst[:nt], in_=pxw[:nt],
                        func=mybir.ActivationFunctionType.Copy)
                    nc.sync.dma_start(out=xwhl[i:i + nt, :], in_=xst[:nt, :])

            # ---- stage B: phase-1 scatter (node -> edge) ----------------
            with tc.tile_pool(name=f"pb{rep}", bufs=3) as pb, \
                 tc.tile_pool(name=f"psb{rep}", bufs=2, space="PSUM") as psb:
                zrow = pb.tile([P, OUT_DIM], BF16, tag="zrow", name="zrow")
                nc.vector.memset(zrow[:], 0.0)
                for t in empty1:
                    nc.sync.dma_start(
                        out=m_part[t * ETW:(t + 1) * ETW, :], in_=zrow[:ETW, :])
                for si, (coff, groups) in enumerate(supers1):
                    ck_tot = sum(k for _, k, _ in groups)
                    G = pb.tile([P, ck_tot, OUT_DIM], BF16, tag="G")
                    if not no_gather:
                        nc.gpsimd.dma_gather(
                            G[:, :, :], xwhl[:, :],
                            idxA[:, coff * 8:(coff + ck_tot) * 8],
                            ck_tot * P, ck_tot * P, OUT_DIM,
                            single_packet=single_packet,
                            queue_num=si % nqueues)
                    else:
                        nc.vector.memset(G[:, 0, :], 0.0)
                    for t, kt, loc in groups:
                        S = pb.tile([P, kt * ETW], BF16, tag="S")
                        s_build(S, ohA, coff + loc, kt, ETW)
                        pm = psb.tile([P, OUT_DIM], F32, tag="pm")
                        nkt = 1 if no_scatmm else kt
                        for c in range(nkt):
                            nc.tensor.matmul(
                                out=pm[:ETW], lhsT=S[:, c * ETW:(c + 1) * ETW],
                                rhs=G[:, loc + c, :],
                                start=(c == 0), stop=(c == nkt - 1),
                                skip_group_check=True)
                        mt = pb.tile([P, OUT_DIM], BF16, tag="mt")
                        nc.scalar.activation(
                            out=mt[:ETW], in_=pm[:ETW],
                            func=mybir.ActivationFunctionType.Copy)
                        nc.sync.dma_start(
                            out=m_part[t * ETW:(t + 1) * ETW, :], in_=mt[:ETW, :])

            # ---- stage C: ReduceScatter -> scale -> AllGather -----------
            no_cc = os.environ.get("DBG_NO_CC") == "1"  # TimelineSim can't
            if no_cc:                                   # model collectives
                nc.sync.dma_start(out=m_shard[:, :], in_=m_part[:SHARD, :])
            else:
                nc.gpsimd.collective_compute(
                    "ReduceScatter", mybir.AluOpType.add,
                    replica_groups=[list(range(NCORES))],
                    ins=[m_part[:, :]], outs=[m_shard[:, :]])
            with tc.tile_pool(name=f"pc{rep}", bufs=3) as pc:
                for t in range(SHARD // P):
                    ms = pc.tile([P, OUT_DIM], BF16, tag="ms")
                    nc.sync.dma_start(out=ms[:], in_=m_shard[t * P:(t + 1) * P, :])
                    st = pc.tile([P, OUT_DIM], BF16, tag="st")
                    nc.vector.tensor_scalar(
                        out=st[:], in0=ms[:], scalar1=binv_sb[:, t:t + 1],
                        scalar2=None, op0=mybir.AluOpType.mult)
                    nc.sync.dma_start(out=mtab_sh[t * P:(t + 1) * P, :], in_=st[:])
            if no_cc:
                for cc in range(NCORES):
                    nc.sync.dma_start(
                        out=mtab[cc * SHARD:(cc + 1) * SHARD, :],
                        in_=mtab_sh[:, :])
            else:
                nc.gpsimd.collective_compute(
                    "AllGather", mybir.AluOpType.bypass,
                    replica_groups=[list(range(NCORES))],
                    ins=[mtab_sh[:, :]], outs=[mtab[:, :]])

            # ---- stage D: phase-2 scatter (edge -> node), transposed ----
            with tc.tile_pool(name=f"pd{rep}", bufs=3) as pd, \
                 tc.tile_pool(name=f"psd{rep}", bufs=2, space="PSUM") as psd:
                for si, (coff, groups) in enumerate(supers2):
                    ck_tot = sum(k for _, k, _ in groups)
                    G2 = pd.tile([P, ck_tot, OUT_DIM], BF16, tag="G2")
                    if not no_gather:
                        nc.gpsimd.dma_gather(
                            G2[:, :, :], mtab[:, :],
                            idxB[:, coff * 8:(coff + ck_tot) * 8],
                            ck_tot * P, ck_tot * P, OUT_DIM,
                            single_packet=single_packet,
                            queue_num=si % nqueues)
                    else:
                        nc.vector.memset(G2[:, 0, :], 0.0)
                    for gg, kt, loc in groups:
                        S2 = pd.tile([P, kt * NTW], BF16, tag="S2")
                        s_build(S2, ohB, coff + loc, kt, NTW)
                        poT = psd.tile([P, NTW], F32, tag="poT")
                        nkt = 1 if no_scatmm else kt
                        for c in range(nkt):
                            nc.tensor.matmul(
                                out=poT[:], lhsT=G2[:, loc + c, :],
                                rhs=S2[:, c * NTW:(c + 1) * NTW],
                                start=(c == 0), stop=(c == nkt - 1),
                                skip_group_check=True)
                        ot = pd.tile([P, NTW], F32, tag="ot")
                        nc.vector.tensor_tensor(
                            out=ot[:], in0=poT[:],
                            in1=dinv_bc[:, gg * NTW:(gg + 1) * NTW],
                            op=mybir.AluOpType.mult)
                        nc.scalar.activation(
                            out=accT[:, gg * NTW:(gg + 1) * NTW], in_=ot[:],
                            func=mybir.ActivationFunctionType.Relu,
                            bias=bias_sb[:, 0:1])
                # zero phantom-node columns (beyond npc) before the reduce
                if NROWS > npc:
                    nc.vector.memset(accT[:, npc:NROWS], 0.0)

            # ---- stage E: row-sum over all node columns -> [OUT_DIM, 1] -
            with tc.tile_pool(name=f"pe{rep}", bufs=1) as pe:
                osum = pe.tile([P, 1], F32)
                nc.vector.tensor_reduce(
                    out=osum[:], in_=accT[:], axis=mybir.AxisListType.X,
                    op=mybir.AluOpType.add)
                nc.sync.dma_start(out=out_part[:, :], in_=osum[:OUT_DIM])

    nc.compile()
    return nc


def prepare_inputs(x, w, bias, hyperedge_index):
    """Host-side sharding: split entries by src-node shard, bucket/pad both
    phase streams, compute degrees + static chunk structure (shared by all
    cores)."""
    npc, n_node_groups, n_edge_groups, EROWS, SHARD = _derived()
    NROWS = n_node_groups * NTW
    src = np.asarray(hyperedge_index[0], dtype=np.int64)
    edge = np.asarray(hyperedge_index[1], dtype=np.int64)

    # exact degree reciprocals (host)
    deg_e = np.bincount(edge, minlength=N_EDGES).astype(np.float64)
    b_inv_full = np.zeros(EROWS, np.float32)
    nzmask = deg_e > 0
    b_inv_full[:N_EDGES][nzmask] = (1.0 / deg_e[nzmask]).astype(np.float32)

    core_of = src // npc
    per_core = []
    for c in range(NCORES):
        sel = core_of == c
        per_core.append((src[sel] - c * npc, edge[sel]))

    cnt1 = np.zeros((NCORES, n_edge_groups), np.int64)
    cnt2 = np.zeros((NCORES, n_node_groups), np.int64)
    for c, (s_loc, e_glob) in enumerate(per_core):
        cnt1[c] = np.bincount(e_glob // ETW, minlength=n_edge_groups)
        cnt2[c] = np.bincount(s_loc // NTW, minlength=n_node_groups)
    chunks1 = -(-cnt1.max(axis=0) // P)                 # may be 0
    chunks2 = np.maximum(1, -(-cnt2.max(axis=0) // P))  # >= 1 (bias/relu rows)

    in_maps = []
    for c, (s_loc, e_glob) in enumerate(per_core):
        g1, oh1 = _bucket_entries(
            s_loc, (e_glob % ETW).astype(np.float32), e_glob // ETW,
            n_edge_groups, chunks1)
        g2, oh2 = _bucket_entries(
            e_glob, (s_loc % NTW).astype(np.float32), s_loc // NTW,
            n_node_groups, chunks2)

        deg_n = np.bincount(s_loc, minlength=npc).astype(np.float64)
        d_inv = np.zeros(NROWS, np.float32)
        nz = deg_n > 0
        d_inv[:npc][nz] = (1.0 / deg_n[nz]).astype(np.float32)

        binv_shard = np.ascontiguousarray(
            b_inv_full[c * SHARD:(c + 1) * SHARD].reshape(SHARD // P, P).T)

        xT = np.ascontiguousarray(
            x[c * npc:(c + 1) * npc].T.astype(ml_dtypes.bfloat16))
        in_maps.append({
            "xT": xT,
            "w": np.ascontiguousarray(w.astype(ml_dtypes.bfloat16)),
            "biasT": np.ascontiguousarray(bias.astype(np.float32)).reshape(-1, 1),
            "dinv": d_inv.reshape(1, -1),
            "binv": binv_shard,
            "idxA": _wrap_idx16(g1),
            "ohA": _oh_cols(oh1),
            "idxB": _wrap_idx16(g2),
            "ohB": _oh_cols(oh2),
        })

    return in_maps, chunks1, chunks2


def kernel(x_node_features, lin_weight, bias, hyperedge_index):
    in_maps, chunks1, chunks2 = prepare_inputs(
        x_node_features, lin_weight, bias, hyperedge_index)
    nc = build_kernel(chunks1, chunks2)
    res = run_bass_kernel_spmd(nc, in_maps, list(range(NCORES)))
    total = np.zeros(OUT_DIM, np.float64)
    for c in range(NCORES):
        total += res.results[c]["out_part"][:, 0].astype(np.float64)
    return (total / N_NODES).astype(np.float32)


# revision 27
# speedup vs baseline: 2.2068x; 1.1779x over previous
"""HypergraphConv (node->edge->node message passing) on 8 Trainium2 NeuronCores.

Self-contained Trainium kernel for:
    xw   = x @ W
    m_e  = (1/deg_e) * sum_{k: edge[k]=e} xw[src[k]]
    o_i  = (1/deg_i) * sum_{k: src[k]=i} m_{edge[k]} + bias
    out  = mean_i relu(o_i)                       # [128]

Sharding: nodes are split across the 8 cores (6250 each). Each core owns the
incidence entries whose src node falls in its shard; those entries drive both
the node->edge scatter (partial m, ReduceScattered + AllGathered across cores)
and the edge->node scatter (complete rows for the core's nodes).

Scatters are one-hot matmuls over sorted-and-padded entry streams grouped into
128-wide edge tiles / 64-wide node tiles; gathers use the SWDGE dma_gather
engine against bf16 row tables (256-byte rows). Degrees are computed
host-side and shipped as small inputs. The phase-2 matmul is transposed
(out = G^T-style [feat, node]) so bias+ReLU fuse into one Activation-engine op
writing straight into the accumulation buffer.
"""

import os
import numpy as np
import ml_dtypes
from contextlib import ExitStack

import concourse.bacc as bacc
import concourse.bass as bass
import concourse.mybir as mybir
import concourse.tile as tile
from concourse import library_config
from concourse.bass_utils import run_bass_kernel_spmd

NCORES = 8
P = 128

N_NODES = 50000
N_EDGES = 20000
IN_DIM = 256
OUT_DIM = 128

ETW = 128          # edge-tile (group) width for phase-1 scatter
NTW = 64           # node-tile width for phase-2 scatter
SUPER = int(os.environ.get("KSUPER", "48"))  # 128-entry chunks per gather call

BF16 = mybir.dt.bfloat16
F32 = mybir.dt.float32
I16 = mybir.dt.int16

PAD_OH = 200.0  # one-hot index for padding entries: matches no iota column


def _derived():
    npc = N_NODES // NCORES
    n_node_groups = (npc + NTW - 1) // NTW
    quantum = NCORES * P
    erows = -(-N_EDGES // quantum) * quantum  # RS-shardable, mult of 128
    n_edge_groups = erows // ETW
    return npc, n_node_groups, n_edge_groups, erows, erows // NCORES


def _wrap_idx16(idx):
    """[L] int -> [128, L//16] int16 SWDGE index layout (16-wrap, x8 replicas)."""
    a = np.asarray(idx, dtype=np.int16).reshape(-1, 16).T
    return np.ascontiguousarray(np.tile(a, (8, 1)))


def _oh_cols(oh):
    """[L] float -> [128, L//128] bf16: column c holds entries c*128..c*128+127."""
    return np.ascontiguousarray(oh.reshape(-1, P).T.astype(ml_dtypes.bfloat16))


def _bucket_entries(gidx, ohval, tid, n_tiles, chunks):
    """Lay out (gather idx, one-hot) entry streams grouped by tile.

    chunks[t]: number of 128-entry chunks allotted to tile t (static, shared
    across cores). Pad gather idx = 0 (contribution killed by the all-zero
    one-hot row). Returns (gather_idx[L], onehot[L]).
    """
    # sort by (tile, gather idx): idx-sorted slots give the SWDGE's 16-idx
    # descriptors HBM row-buffer locality; slot order is free (one-hot maps it)
    order = np.lexsort((gidx, tid))
    gidx = gidx[order]
    ohval = ohval[order]
    tid_s = tid[order]
    counts = np.bincount(tid_s, minlength=n_tiles)
    starts = np.concatenate([[0], np.cumsum(counts[:-1])])
    dest_base = np.concatenate([[0], np.cumsum(chunks[:-1])]) * P
    L = int(chunks.sum()) * P
    g_out = np.zeros(L, dtype=np.int64)
    oh_out = np.full(L, PAD_OH, dtype=np.float32)
    n = gidx.shape[0]
    rank = np.arange(n, dtype=np.int64) - starts[tid_s]
    dest = dest_base[tid_s] + rank
    g_out[dest] = gidx
    oh_out[dest] = ohval
    return g_out, oh_out


def build_kernel(chunks1, chunks2):
    """Build the SPMD device program.

    chunks1[g]: #chunks for edge group g (phase 1, may be 0);
    chunks2[gg]: #chunks for node group gg (phase 2, >= 1).
    """
    npc, n_node_groups, n_edge_groups, EROWS, SHARD = _derived()
    NROWS = n_node_groups * NTW
    LA = int(np.sum(chunks1)) * P
    LB = int(np.sum(chunks2)) * P
    NCA = LA // P
    NCB = LB // P
    REPS = int(os.environ.get("KREPS", "1"))
    no_gather = os.environ.get("DBG_NO_GATHER") == "1"  # timing bisection only
    no_scatmm = os.environ.get("DBG_NO_SCATMM") == "1"
    single_packet = os.environ.get("KSP", "0") == "1"
    scratch = int(os.environ.get("KSCRATCH", "32768"))
    nqueues = int(os.environ.get("KQUEUES", "4"))  # ucode MAX_SWDGE_QUEUES=4

    nc = bacc.Bacc("TRN2", num_devices=NCORES,
                   dynamic_dma_scratch_size=scratch,
                   num_swdge_queues=nqueues)

    xT_in = nc.dram_tensor("xT", [IN_DIM, npc], BF16, kind="ExternalInput")
    w_in = nc.dram_tensor("w", [IN_DIM, OUT_DIM], BF16, kind="ExternalInput")
    biasT_in = nc.dram_tensor("biasT", [OUT_DIM, 1], F32, kind="ExternalInput")
    dinv_in = nc.dram_tensor("dinv", [1, NROWS], F32, kind="ExternalInput")
    binv_in = nc.dram_tensor("binv", [P, SHARD // P], F32, kind="ExternalInput")
    idxA_in = nc.dram_tensor("idxA", [P, LA // 16], I16, kind="ExternalInput")
    ohA_in = nc.dram_tensor("ohA", [P, NCA], BF16, kind="ExternalInput")
    idxB_in = nc.dram_tensor("idxB", [P, LB // 16], I16, kind="ExternalInput")
    ohB_in = nc.dram_tensor("ohB", [P, NCB], BF16, kind="ExternalInput")
    out_part = nc.dram_tensor("out_part", [OUT_DIM, 1], F32, kind="ExternalOutput")

    xwhl = nc.dram_tensor("xwhl", [npc, OUT_DIM], BF16)
    m_part = nc.dram_tensor("m_part", [EROWS, OUT_DIM], BF16)
    m_shard = nc.dram_tensor("m_shard", [SHARD, OUT_DIM], BF16)
    mtab_sh = nc.dram_tensor("mtab_sh", [SHARD, OUT_DIM], BF16)
    mtab = nc.dram_tensor("mtab", [EROWS, OUT_DIM], BF16, addr_space="Shared")

    # phase-1 super-groups: consecutive edge groups gathered in one call
    def make_supers(chunks):
        supers = []  # (chunk_offset, [(tile_idx, kt, local_chunk_off)])
        cur, ck, coff = [], 0, 0
        base = 0
        for t, k in enumerate(chunks):
            k = int(k)
            if k == 0:
                continue
            if ck + k > SUPER and cur:
                supers.append((coff, cur))
                coff += ck
                cur, ck = [], 0
            cur.append((t, k, ck))
            ck += k
        if cur:
            supers.append((coff, cur))
        return supers

    supers1 = make_supers(chunks1)
    supers2 = make_supers(chunks2)
    empty1 = [t for t, k in enumerate(chunks1) if int(k) == 0]

    with tile.TileContext(nc) as tc, ExitStack() as ctx:
        pin = ctx.enter_context(tc.tile_pool(name="pin", bufs=1))

        nc.gpsimd.load_library(library_config.mlp)

        # ---- persistent small tiles (once, outside reps) ----------------
        iota_i = pin.tile([P, P], I16)
        iota_bf = pin.tile([P, P], BF16)
        nc.gpsimd.iota(iota_i[:], [[1, P]], channel_multiplier=0)
        nc.vector.tensor_copy(out=iota_bf[:], in_=iota_i[:])

        def s_build(S_tile, oh_tile, col0, k, w):
            """S[p, c*w+j] = (oh[p, col0+c] == j), one DVE op for k chunks."""
            s_ap = S_tile[:].rearrange("p (k j) -> p k j", k=k)
            o = oh_tile[:, col0:col0 + k]
            in0 = bass.AP(o.tensor, o.offset, [list(o.ap[0]), list(o.ap[1]), [0, w]])
            it = iota_bf[:]
            in1 = bass.AP(it.tensor, it.offset, [list(it.ap[0]), [0, k], [1, w]])
            nc.vector.tensor_tensor(out=s_ap, in0=in0, in1=in1, op=mybir.AluOpType.is_equal)

        for rep in range(REPS):
          with tc.tile_pool(name=f"prep{rep}", bufs=1) as pr:
            # streams + per-rep persistent tiles
            idxA = pr.tile([P, LA // 16], I16, name="idxA")
            ohA = pr.tile([P, NCA], BF16, name="ohA")
            idxB = pr.tile([P, LB // 16], I16, name="idxB")
            ohB = pr.tile([P, NCB], BF16, name="ohB")
            nc.sync.dma_start(out=idxA[:], in_=idxA_in[:])
            nc.sync.dma_start(out=ohA[:], in_=ohA_in[:])
            nc.sync.dma_start(out=idxB[:], in_=idxB_in[:])
            nc.sync.dma_start(out=ohB[:], in_=ohB_in[:])
            bias_sb = pr.tile([P, 1], F32, name="biasT")
            nc.sync.dma_start(out=bias_sb[:OUT_DIM], in_=biasT_in[:])
            dinv_bc = pr.tile([P, NROWS], F32, name="dinvbc")
            nc.sync.dma_start(
                out=dinv_bc[:], in_=bass.AP(dinv_in, 0, [[0, P], [1, NROWS]]))
            binv_sb = pr.tile([P, SHARD // P], F32, name="binv")
            nc.sync.dma_start(out=binv_sb[:], in_=binv_in[:])
            accT = pr.tile([P, NROWS], F32, name="accT")

            # ---- stage A: xw = x @ W -> bf16 row table ------------------
            with tc.tile_pool(name=f"pa{rep}", bufs=1) as pa, \
                 tc.tile_pool(name=f"pa2{rep}", bufs=3) as pa2, \
                 tc.tile_pool(name=f"psa{rep}", bufs=2, space="PSUM") as psa:
                kh = IN_DIM // P
                xT_sb = [pa.tile([P, npc], BF16, name=f"xT{k}") for k in range(kh)]
                w_sb = [pa.tile([P, OUT_DIM], BF16, name=f"wsb{k}") for k in range(kh)]
                for k in range(kh):
                    nc.sync.dma_start(out=xT_sb[k][:], in_=xT_in[k * P:(k + 1) * P, :])
                    nc.sync.dma_start(out=w_sb[k][:], in_=w_in[k * P:(k + 1) * P, :])
                for i in range(0, npc, P):
                    nt = min(P, npc - i)
                    pxw = psa.tile([P, OUT_DIM], F32, tag="pxw")
                    for k in range(kh):
                        nc.tensor.matmul(
                            out=pxw[:nt], lhsT=xT_sb[k][:, i:i + nt], rhs=w_sb[k][:],
                            start=(k == 0), stop=(k == kh - 1))
                    xst = pa2.tile([P, OUT_DIM], BF16, tag="xst")
                    nc.scalar.activation(
                        out=xst[:nt], in_=pxw[:nt],
                        func=mybir.ActivationFunctionType.Copy)
                    nc.sync.dma_start(out=xwhl[i:i + nt, :], in_=xst[:nt, :])

            # ---- stage B: phase-1 scatter (node -> edge) ----------------
            gbufs = int(os.environ.get("KBUFS", "4"))
            with tc.tile_pool(name=f"pb{rep}", bufs=gbufs) as pb, \
                 tc.tile_pool(name=f"psb{rep}", bufs=2, space="PSUM") as psb:
                zrow = pb.tile([P, OUT_DIM], BF16, tag="zrow", name="zrow")
                nc.vector.memset(zrow[:], 0.0)
                for t in empty1:
                    nc.sync.dma_start(
                        out=m_part[t * ETW:(t + 1) * ETW, :], in_=zrow[:ETW, :])
                for si, (coff, groups) in enumerate(supers1):
                    ck_tot = sum(k for _, k, _ in groups)
                    G = pb.tile([P, ck_tot, OUT_DIM], BF16, tag="G")
                    if not no_gather:
                        nc.gpsimd.dma_gather(
                            G[:, :, :], xwhl[:, :],
                            idxA[:, coff * 8:(coff + ck_tot) * 8],
                            ck_tot * P, ck_tot * P, OUT_DIM,
                            single_packet=single_packet,
                            queue_num=si % nqueues)
                    else:
                        nc.vector.memset(G[:, 0, :], 0.0)
                    for t, kt, loc in groups:
                        S = pb.tile([P, kt * ETW], BF16, tag="S")
                        s_build(S, ohA, coff + loc, kt, ETW)
                        pm = psb.tile([P, OUT_DIM], F32, tag="pm")
                        nkt = 1 if no_scatmm else kt
                        for c in range(nkt):
                            nc.tensor.matmul(
                                out=pm[:ETW], lhsT=S[:, c * ETW:(c + 1) * ETW],
                                rhs=G[:, loc + c, :],
                                start=(c == 0), stop=(c == nkt - 1),
                                skip_group_check=True)
                        mt = pb.tile([P, OUT_DIM], BF16, tag="mt")
                        nc.scalar.activation(
                            out=mt[:ETW], in_=pm[:ETW],
                            func=mybir.ActivationFunctionType.Copy)
                        nc.sync.dma_start(
                            out=m_part[t * ETW:(t + 1) * ETW, :], in_=mt[:ETW, :])

            # ---- stage C: ReduceScatter -> scale -> AllGather -----------
            no_cc = os.environ.get("DBG_NO_CC") == "1"  # TimelineSim can't
            if no_cc:                                   # model collectives
                nc.sync.dma_start(out=m_shard[:, :], in_=m_part[:SHARD, :])
            else:
                nc.gpsimd.collective_compute(
                    "ReduceScatter", mybir.AluOpType.add,
                    replica_groups=[list(range(NCORES))],
                    ins=[m_part[:, :]], outs=[m_shard[:, :]])
            with tc.tile_pool(name=f"pc{rep}", bufs=3) as pc:
                for t in range(SHARD // P):
                    ms = pc.tile([P, OUT_DIM], BF16, tag="ms")
                    nc.sync.dma_start(out=ms[:], in_=m_shard[t * P:(t + 1) * P, :])
                    st = pc.tile([P, OUT_DIM], BF16, tag="st")
                    nc.vector.tensor_scalar(
                        out=st[:], in0=ms[:], scalar1=binv_sb[:, t:t + 1],
                        scalar2=None, op0=mybir.AluOpType.mult)
                    nc.sync.dma_start(out=mtab_sh[t * P:(t + 1) * P, :], in_=st[:])
            if no_cc:
                for cc in range(NCORES):
                    nc.sync.dma_start(
                        out=mtab[cc * SHARD:(cc + 1) * SHARD, :],
                        in_=mtab_sh[:, :])
            else:
                nc.gpsimd.collective_compute(
                    "AllGather", mybir.AluOpType.bypass,
                    replica_groups=[list(range(NCORES))],
                    ins=[mtab_sh[:, :]], outs=[mtab[:, :]])

            # ---- stage D: phase-2 scatter (edge -> node), transposed ----
            with tc.tile_pool(name=f"pd{rep}", bufs=gbufs) as pd, \
                 tc.tile_pool(name=f"psd{rep}", bufs=2, space="PSUM") as psd:
                for si, (coff, groups) in enumerate(supers2):
                    ck_tot = sum(k for _, k, _ in groups)
                    G2 = pd.tile([P, ck_tot, OUT_DIM], BF16, tag="G2")
                    if not no_gather:
                        nc.gpsimd.dma_gather(
                            G2[:, :, :], mtab[:, :],
                            idxB[:, coff * 8:(coff + ck_tot) * 8],
                            ck_tot * P, ck_tot * P, OUT_DIM,
                            single_packet=single_packet,
                            queue_num=si % nqueues)
                    else:
                        nc.vector.memset(G2[:, 0, :], 0.0)
                    for gg, kt, loc in groups:
                        S2 = pd.tile([P, kt * NTW], BF16, tag="S2")
                        s_build(S2, ohB, coff + loc, kt, NTW)
                        poT = psd.tile([P, NTW], F32, tag="poT")
                        nkt = 1 if no_scatmm else kt
                        for c in range(nkt):
                            nc.tensor.matmul(
                                out=poT[:], lhsT=G2[:, loc + c, :],
                                rhs=S2[:, c * NTW:(c + 1) * NTW],
                                start=(c == 0), stop=(c == nkt - 1),
                                skip_group_check=True)
                        ot = pd.tile([P, NTW], F32, tag="ot")
                        nc.vector.tensor_tensor(
                            out=ot[:], in0=poT[:],
                            in1=dinv_bc[:, gg * NTW:(gg + 1) * NTW],
                            op=mybir.AluOpType.mult)
                        nc.scalar.activation(
                            out=accT[:, gg * NTW:(gg + 1) * NTW], in_=ot[:],
                            func=mybir.ActivationFunctionType.Relu,
                            bias=bias_sb[:, 0:1])
                # zero phantom-node columns (beyond npc) before the reduce
                if NROWS > npc:
                    nc.vector.memset(accT[:, npc:NROWS], 0.0)

            # ---- stage E: row-sum over all node columns -> [OUT_DIM, 1] -
            with tc.tile_pool(name=f"pe{rep}", bufs=1) as pe:
                osum = pe.tile([P, 1], F32)
                nc.vector.tensor_reduce(
                    out=osum[:], in_=accT[:], axis=mybir.AxisListType.X,
                    op=mybir.AluOpType.add)
                nc.sync.dma_start(out=out_part[:, :], in_=osum[:OUT_DIM])

    nc.compile()
    return nc


def prepare_inputs(x, w, bias, hyperedge_index):
    """Host-side sharding: split entries by src-node shard, bucket/pad both
    phase streams, compute degrees + static chunk structure (shared by all
    cores)."""
    npc, n_node_groups, n_edge_groups, EROWS, SHARD = _derived()
    NROWS = n_node_groups * NTW
    src = np.asarray(hyperedge_index[0], dtype=np.int64)
    edge = np.asarray(hyperedge_index[1], dtype=np.int64)

    # exact degree reciprocals (host)
    deg_e = np.bincount(edge, minlength=N_EDGES).astype(np.float64)
    b_inv_full = np.zeros(EROWS, np.float32)
    nzmask = deg_e > 0
    b_inv_full[:N_EDGES][nzmask] = (1.0 / deg_e[nzmask]).astype(np.float32)

    core_of = src // npc
    per_core = []
    for c in range(NCORES):
        sel = core_of == c
        per_core.append((src[sel] - c * npc, edge[sel]))

    cnt1 = np.zeros((NCORES, n_edge_groups), np.int64)
    cnt2 = np.zeros((NCORES, n_node_groups), np.int64)
    for c, (s_loc, e_glob) in enumerate(per_core):
        cnt1[c] = np.bincount(e_glob // ETW, minlength=n_edge_groups)
        cnt2[c] = np.bincount(s_loc // NTW, minlength=n_node_groups)
    chunks1 = -(-cnt1.max(axis=0) // P)                 # may be 0
    chunks2 = np.maximum(1, -(-cnt2.max(axis=0) // P))  # >= 1 (bias/relu rows)

    in_maps = []
    for c, (s_loc, e_glob) in enumerate(per_core):
        g1, oh1 = _bucket_entries(
            s_loc, (e_glob % ETW).astype(np.float32), e_glob // ETW,
            n_edge_groups, chunks1)
        g2, oh2 = _bucket_entries(
            e_glob, (s_loc % NTW).astype(np.float32), s_loc // NTW,
            n_node_groups, chunks2)

        deg_n = np.bincount(s_loc, minlength=npc).astype(np.float64)
        d_inv = np.zeros(NROWS, np.float32)
        nz = deg_n > 0
        d_inv[:npc][nz] = (1.0 / deg_n[nz]).astype(np.float32)

        binv_shard = np.ascontiguousarray(
            b_inv_full[c * SHARD:(c + 1) * SHARD].reshape(SHARD // P, P).T)

        xT = np.ascontiguousarray(
            x[c * npc:(c + 1) * npc].T.astype(ml_dtypes.bfloat16))
        in_maps.append({
            "xT": xT,
            "w": np.ascontiguousarray(w.astype(ml_dtypes.bfloat16)),
            "biasT": np.ascontiguousarray(bias.astype(np.float32)).reshape(-1, 1),
            "dinv": d_inv.reshape(1, -1),
            "binv": binv_shard,
            "idxA": _wrap_idx16(g1),
            "ohA": _oh_cols(oh1),
            "idxB": _wrap_idx16(g2),
            "ohB": _oh_cols(oh2),
        })

    return in_maps, chunks1, chunks2


def kernel(x_node_features, lin_weight, bias, hyperedge_index):
    in_maps, chunks1, chunks2 = prepare_inputs(
        x_node_features, lin_weight, bias, hyperedge_index)
    nc = build_kernel(chunks1, chunks2)
    res = run_bass_kernel_spmd(nc, in_maps, list(range(NCORES)))
    total = np.zeros(OUT_DIM, np.float64)
    for c in range(NCORES):
        total += res.results[c]["out_part"][:, 0].astype(np.float64)
    return (total / N_NODES).astype(np.float32)
